# revision 3
# baseline (speedup 1.0000x reference)
"""Distributed causal multi-head attention (QKV projection + flash attention)
for Trainium2, sharded head-parallel across 8 NeuronCores. v11.

Sharding: core c handles batch c//4 and the 4 heads 4*(c%4)..4*(c%4)+3.
Each core's output slice is disjoint -> no collectives.

v5 changes vs v1 (all-bf16 -- every fp8 variant fails the 2e-2 budget;
gpsimd cannot touch PSUM, so PSUM-side ops stay on DVE):
  - batched DMAs: one descriptor per chunk for x, one per weight tensor
  - scores/exp in shaped [128, 2, 512] tiles, causally aligned: one Exp
    instruction per block (vs up to 3), no uninitialized-PSUM reads
  - const/qkv tile pools double-buffered so consecutive iterations overlap
    (the next call's projection starts while attention tail still runs)
  - fewer instructions, same work: both heads' tri-mask adds fused into one
    3D-AP DVE op; both heads' PV accumulators in one 2-bank PSUM tile with a
    single output copy + single DMA per pair-chunk
"""

import numpy as np

NUM_HEAD = 16
HEAD_DIM = 64
HIDDEN = 1024
B, S = 2, 2048
N_CORES = 8
HPC = 4          # heads per core
NCH = 4          # sq chunks of 512
CHW = 512        # chunk width
NT = 16          # sk tiles of 128
KB = 8           # k-dim blocks of 128
NEG = -1.0e9
SCALE = HEAD_DIM ** -0.5
SX = 8.0         # host scale on x before fp8 cast
SW = 16.0        # host scale on W before fp8 cast
GAM = SX * SW    # qkv arrive scaled by GAM on device

PROJ_FP8 = False

_CACHE = {}


def _build(repeat=1):
    import concourse.bacc as bacc
    import concourse.mybir as mybir
    import concourse.tile as tile

    f32 = mybir.dt.float32
    bf16 = mybir.dt.bfloat16
    e4 = mybir.dt.float8e4
    AF = mybir.ActivationFunctionType
    DR = mybir.MatmulPerfMode.DoubleRow

    in_dt = e4 if PROJ_FP8 else bf16
    exp_scale = SCALE / (GAM * GAM) if PROJ_FP8 else SCALE

    nc = bacc.Bacc("TRN2", target_bir_lowering=False, debug=False)

    XT = nc.dram_tensor("XT", [HIDDEN, S], in_dt, kind="ExternalInput")
    WQK = nc.dram_tensor("WQK", [HIDDEN, 512], in_dt, kind="ExternalInput")
    WV = nc.dram_tensor("WV", [HIDDEN, 256], in_dt, kind="ExternalInput")
    BQKT = nc.dram_tensor("BQKT", [128, 4], f32, kind="ExternalInput")
    TRI = nc.dram_tensor("TRI", [128, 2, 128], f32, kind="ExternalInput")
    OUT = nc.dram_tensor("OUT", [HPC, 65, S], f32, kind="ExternalOutput")

    with tile.TileContext(nc) as tc:
        with tc.tile_pool(name="const", bufs=2) as const_pool, \
             tc.tile_pool(name="qkv", bufs=2) as qkv_pool, \
             tc.tile_pool(name="xt", bufs=4) as xt_pool, \
             tc.tile_pool(name="exps", bufs=10) as exp_pool, \
             tc.tile_pool(name="outs", bufs=4) as out_pool, \
             tc.tile_pool(name="ps_sc", bufs=2, space="PSUM") as ps_sc, \
             tc.tile_pool(name="ps_pr", bufs=2, space="PSUM") as ps_pr, \
             tc.tile_pool(name="ps_pv", bufs=1, space="PSUM") as ps_pv:

            for _rep in range(repeat):
                wqk_sb = const_pool.tile([128, KB, 512], in_dt, tag="wqk")
                wv_sb = const_pool.tile([128, KB, 256], in_dt, tag="wv")
                bqk_sb = const_pool.tile([128, 4], f32, tag="bqk")
                tri_sb = const_pool.tile([128, 2, 128], f32, tag="tri")

                nc.gpsimd.dma_start(
                    wqk_sb[:], WQK[:].rearrange("(a p) c -> p a c", p=128))
                nc.gpsimd.dma_start(
                    wv_sb[:], WV[:].rearrange("(a p) c -> p a c", p=128))
                nc.sync.dma_start(bqk_sb[:], BQKT[:])
                nc.sync.dma_start(tri_sb[:], TRI[:])

                # qT2/kT2: [pair, 128 (2 heads x 64 d), S]; v: [sk-tile, head, 65]
                qT2 = qkv_pool.tile([128, 2, S], bf16, tag="qT2")
                kT2 = qkv_pool.tile([128, 2, S], bf16, tag="kT2")
                v_sb = qkv_pool.tile([128, NT, HPC, 65], bf16, tag="v")
                nc.vector.memset(v_sb[:, :, :, 64], 1.0)

                def emit_xt_dma(C):
                    xt = xt_pool.tile([128, KB, CHW], in_dt, tag="xt")
                    nc.gpsimd.dma_start(
                        xt[:],
                        XT[:, C * CHW:(C + 1) * CHW]
                        .rearrange("(a p) c -> p a c", p=128))
                    return xt

                def emit_qkT_group(C, xt, blk):
                    # col-blocks: 0,1 = q pair0/pair1; 2,3 = k pair0/pair1
                    ps = ps_pr.tile([128, CHW], f32, tag="pr")
                    if PROJ_FP8:
                        for j in range(KB // 2):
                            nc.tensor.matmul(
                                ps[:],
                                wqk_sb[:, 2 * j:2 * j + 2,
                                       blk * 128:(blk + 1) * 128],
                                xt[:, 2 * j:2 * j + 2, :],
                                start=(j == 0), stop=(j == KB // 2 - 1),
                                perf_mode=DR)
                    else:
                        for kb in range(KB):
                            nc.tensor.matmul(
                                ps[:],
                                wqk_sb[:, kb, blk * 128:(blk + 1) * 128],
                                xt[:, kb, :],
                                start=(kb == 0), stop=(kb == KB - 1))
                    dest = (qT2 if blk < 2 else kT2)[:, blk % 2,
                                                     C * CHW:(C + 1) * CHW]
                    nc.vector.tensor_scalar_add(dest, ps[:],
                                                bqk_sb[:, blk:blk + 1])

                def emit_v_group(C, xt, rt):
                    t = C * 4 + rt
                    psv = ps_pr.tile([128, 256], f32, tag="pr")
                    if PROJ_FP8:
                        for j in range(KB // 2):
                            nc.tensor.matmul(
                                psv[:],
                                xt[:, 2 * j:2 * j + 2, rt * 128:(rt + 1) * 128],
                                wv_sb[:, 2 * j:2 * j + 2, :],
                                start=(j == 0), stop=(j == KB // 2 - 1),
                                perf_mode=DR)
                    else:
                        for kb in range(KB):
                            nc.tensor.matmul(
                                psv[:],
                                xt[:, kb, rt * 128:(rt + 1) * 128],
                                wv_sb[:, kb, :],
                                start=(kb == 0), stop=(kb == KB - 1))
                    nc.vector.tensor_copy(v_sb[:, t, :, 0:64], psv[:])

                def proj_pair(C, xt, p):
                    emit_qkT_group(C, xt, p)
                    emit_qkT_group(C, xt, 2 + p)
                    if p == 0:
                        for rt in range(4):
                            emit_v_group(C, xt, rt)

                for C in range(NCH):
                    xt_c = emit_xt_dma(C)

                    # ---- attention for sq chunk C, both head pairs ----
                    for p in range(2):
                        proj_pair(C, xt_c, p)
                        hA, hB = 2 * p, 2 * p + 1
                        pvAB = ps_pv.tile([128, 2, CHW], f32, tag="pv")
                        nblk = 4 * C + 4

                        def emit_qk(i):
                            m = i - 4 * C
                            off = 0 if m < 0 else 128 * m
                            w = CHW - off
                            sqs = C * CHW + off
                            psM = ps_sc.tile([128, 2, CHW], f32, tag="sc")
                            nc.tensor.matmul(
                                psM[:, 0, off:CHW],
                                kT2[0:64, p, i * 128:(i + 1) * 128],
                                qT2[0:64, p, sqs:sqs + w],
                                start=True, stop=True, tile_position=(0, 0))
                            nc.tensor.matmul(
                                psM[:, 1, off:CHW],
                                kT2[64:128, p, i * 128:(i + 1) * 128],
                                qT2[64:128, p, sqs:sqs + w],
                                start=True, stop=True, tile_position=(64, 0))
                            return psM, m, off, w

                        def emit_tail(i, psM, m, off, w):
                            expM = exp_pool.tile([128, 2, CHW], bf16, tag="exp")
                            if m >= 0:
                                # causal mask, both heads in one 3D-AP op
                                nc.vector.tensor_add(psM[:, :, off:off + 128],
                                                     psM[:, :, off:off + 128],
                                                     tri_sb[:])
                            nc.scalar.activation(expM[:, :, off:CHW],
                                                 psM[:, :, off:CHW],
                                                 AF.Exp, scale=exp_scale)
                            nc.tensor.matmul(
                                pvAB[0:65, 0, off:CHW], v_sb[:, i, hA, :],
                                expM[:, 0, off:CHW],
                                start=(i == 0), stop=(i == nblk - 1))
                            nc.tensor.matmul(
                                pvAB[0:65, 1, off:CHW], v_sb[:, i, hB, :],
                                expM[:, 1, off:CHW],
                                start=(i == 0), stop=(i == nblk - 1))

                        pending = None
                        for i in range(nblk):
                            cur = emit_qk(i)
                            if pending is not None:
                                emit_tail(i - 1, *pending)
                            pending = cur
                        emit_tail(nblk - 1, *pending)
                        oAB = out_pool.tile([128, 2, CHW], f32, tag="o")
                        nc.vector.tensor_copy(oAB[0:65, :, :], pvAB[0:65, :, :])
                        nc.sync.dma_start(
                            OUT[hA:hA + 2, :, C * CHW:(C + 1) * CHW]
                            .rearrange("h p c -> p h c"),
                            oAB[0:65, :, :])

    nc.compile()
    return nc


def _get_nc(repeat=1):
    key = ("nc", repeat)
    if key not in _CACHE:
        _CACHE[key] = _build(repeat)
    return _CACHE[key]


def _prep_inputs(x, W, b):
    import ml_dtypes
    bf16 = ml_dtypes.bfloat16
    e4 = ml_dtypes.float8_e4m3
    in_np = e4 if PROJ_FP8 else bf16
    bias_scale = GAM if PROJ_FP8 else 1.0

    x = np.asarray(x, dtype=np.float32)
    W = np.asarray(W, dtype=np.float32)
    b = np.asarray(b, dtype=np.float32)

    W4 = W.reshape(HIDDEN, 3, NUM_HEAD, HEAD_DIM)
    b4 = b.reshape(3, NUM_HEAD, HEAD_DIM)

    if PROJ_FP8:
        xT = [np.ascontiguousarray((x[bi] * SX).T).astype(in_np)
              for bi in range(B)]
        Wsc = SW
    else:
        xT = [np.ascontiguousarray(x[bi].T).astype(in_np) for bi in range(B)]
        Wsc = 1.0

    tri1 = np.where(np.arange(128)[None, :] >= np.arange(128)[:, None],
                    np.float32(0.0), np.float32(NEG)).astype(np.float32)
    tri = np.ascontiguousarray(np.stack([tri1, tri1], axis=1))

    in_maps = []
    for c in range(N_CORES):
        bi, g = divmod(c, HPC)
        heads = [4 * g + j for j in range(HPC)]
        wqk = np.concatenate(
            [W4[:, 0, h, :] for h in heads] + [W4[:, 1, h, :] for h in heads],
            axis=1)  # [1024, 512]
        wv = np.concatenate([W4[:, 2, h, :] for h in heads], axis=1)  # [1024,256]
        bqkt = np.stack(
            [np.concatenate([b4[0, heads[0]], b4[0, heads[1]]]),
             np.concatenate([b4[0, heads[2]], b4[0, heads[3]]]),
             np.concatenate([b4[1, heads[0]], b4[1, heads[1]]]),
             np.concatenate([b4[1, heads[2]], b4[1, heads[3]]])],
            axis=1) * bias_scale  # [128, 4]
        in_maps.append({
            "XT": xT[bi],
            "WQK": np.ascontiguousarray(wqk * Wsc).astype(in_np),
            "WV": np.ascontiguousarray(wv * Wsc).astype(in_np),
            "BQKT": np.ascontiguousarray(bqkt),
            "TRI": tri,
        })
    return in_maps, b4


def kernel(x, W, b):
    from concourse.bass_utils import run_bass_kernel_spmd

    in_maps, b4 = _prep_inputs(x, W, b)
    nc = _get_nc()
    res = run_bass_kernel_spmd(nc, in_maps, core_ids=list(range(N_CORES)))

    inv = 1.0 / GAM if PROJ_FP8 else 1.0
    out = np.empty((B, S, NUM_HEAD, HEAD_DIM), dtype=np.float32)
    for c in range(N_CORES):
        bi, g = divmod(c, HPC)
        u = res.results[c]["OUT"]               # [4, 65, 2048]
        o = u[:, :64, :] / u[:, 64:65, :] * inv  # [4, 64, 2048]
        out[bi, :, 4 * g:4 * g + 4, :] = o.transpose(2, 0, 1)
    out += b4[2].reshape(1, 1, NUM_HEAD, HEAD_DIM)
    return out
